# revision 18
# baseline (speedup 1.0000x reference)
"""Trainium2 Bass kernel for nn_DisLoss: dispersion loss over prototype pairs with
a sequential per-sample EMA prototype update.

Strategy (8 NeuronCores, full inputs in / full output out):
  * EMA grouped by label into K rounds (independent across classes). Round 0
    batch-updates every touched label; multi-occurrence labels get a dedicated
    [128,512] tile whose rounds run fully in SBUF with copy_predicated blending.
  * The gram-column operand is split into 8 range tensors p_w{r} [1025,512]
    (range r = classes [1024r,1024(r+1))): indirect-scatter cost scales with the
    DRAM-side AP size, so 1025-row targets are 8x cheaper than one [8193,512]
    tensor, and per-range scatters unlock scatter->transpose->matmul overlap.
  * Row-sharded gram: per core, 512 accumulating fp16 matmuls (fp32 PSUM) over
    DMA-transposed operands; scalar engine fuses exp(10x) with per-row accum_out.
    Host sums per-core partials, subtracts exact exp(1/T) diagonal, log-means.
"""
import sys
import math
import numpy as np

sys.path.insert(0, "/opt/trn_rl_repo")

from concourse import bass, mybir  # noqa: E402
import concourse.tile as tile  # noqa: E402
from concourse.bass import _add_dep_helper  # noqa: E402
from concourse.bass_utils import run_bass_kernel_spmd  # noqa: E402

NCLS, FEAT, BATCH = 8192, 512, 1024
NCORES, P, G = 8, 128, 8
SHARD = NCLS // NCORES
RB = SHARD // P
CT = NCLS // 512
KC = FEAT // P
TEMP = 0.1
BASE_TEMP = 0.1

F16 = mybir.dt.float16
F32 = mybir.dt.float32
I32 = mybir.dt.int32
AF = mybir.ActivationFunctionType
OP = mybir.AluOpType

_PROGRAM_CACHE: dict = {}


def _nr_refine(nc, rinv, tmp, ss, iters=2):
    for _ in range(iters):
        nc.vector.tensor_tensor(out=tmp[:], in0=rinv[:], in1=rinv[:], op=OP.mult)
        nc.vector.tensor_tensor(out=tmp[:], in0=tmp[:], in1=ss[:], op=OP.mult)
        nc.vector.tensor_scalar(out=tmp[:], in0=tmp[:], scalar1=-0.5, scalar2=1.5,
                                op0=OP.mult, op1=OP.add)
        nc.vector.tensor_tensor(out=rinv[:], in0=rinv[:], in1=tmp[:], op=OP.mult)


def _split_multi_waits(nc) -> int:
    """Walrus accepts at most ONE sync-wait per instruction: hoist extras into
    EventSemaphore instructions spliced before the op on the same engine."""
    n = 0
    for f in nc.m.functions:
        for b in f.blocks:
            insts = b.instructions
            i = 0
            while i < len(insts):
                inst = insts[i]
                si = getattr(inst, "sync_info", None)
                if si is not None and si.on_wait is not None and len(si.on_wait) > 1:
                    waits = list(si.on_wait)
                    evs = []
                    for w in waits[:-1]:
                        n += 1
                        evs.append(mybir.InstEventSemaphore(
                            name=f"{inst.name}-ws{n}", engine=inst.engine,
                            ins=[], outs=[],
                            sync_info=mybir.SyncInfo(on_wait=[w], on_update=[])))
                    inst.sync_info = mybir.SyncInfo(
                        on_wait=[waits[-1]], on_update=list(si.on_update or []))
                    insts[i:i] = evs
                    i += len(evs)
                i += 1
    return n


def _ema_norm(nc, pp, sqp, t_ap, tagsfx):
    """ss=sum(t^2); rinv=1/sqrt(ss) (ACT sqrt + DVE recip + 2x NR). [P,1] tiles."""
    sq = sqp.tile([P, FEAT], F16, name=f"sq_{tagsfx}", tag="sqt")
    ss = pp.tile([P, 1], F32, tag=f"ss_{tagsfx}", name=f"ss_{tagsfx}")
    nc.scalar.activation(sq[:], t_ap, AF.Square, accum_out=ss[:, 0:1])
    nrm = pp.tile([P, 1], F32, tag=f"nrm_{tagsfx}", name=f"nrm_{tagsfx}")
    nc.scalar.activation(nrm[:], ss[:], AF.Sqrt)
    rinv = pp.tile([P, 1], F32, tag=f"rinv_{tagsfx}", name=f"rinv_{tagsfx}")
    nc.vector.reciprocal(rinv[:], nrm[:])
    tmp = pp.tile([P, 1], F32, tag=f"tmp_{tagsfx}", name=f"tmp_{tagsfx}")
    _nr_refine(nc, rinv, tmp, ss)
    return rinv


def build_program(K: int, repeat: int = 1, split_waits: bool = True) -> bass.Bass:
    nc = bass.Bass("TRN2", target_bir_lowering=False, debug=False, num_devices=NCORES)
    p_orig = nc.dram_tensor("p_orig", [NCLS + 1, FEAT], F16, kind="ExternalInput").ap()
    p_w = [nc.dram_tensor(f"p_w{r}", [SHARD + 1, FEAT], F16, kind="ExternalInput").ap()
           for r in range(G)]
    p_shard = nc.dram_tensor("p_shard", [SHARD + 1, FEAT], F16, kind="ExternalInput").ap()
    f0 = nc.dram_tensor("f0", [P, G * FEAT], F16, kind="ExternalInput").ap()
    idx0 = nc.dram_tensor("idx0", [P, G], I32, kind="ExternalInput").ap()
    dw = nc.dram_tensor("dw", [P, G], I32, kind="ExternalInput").ap()
    dsh = nc.dram_tensor("dsh", [P, G], I32, kind="ExternalInput").ap()
    idxm = nc.dram_tensor("idxm", [P, 1], I32, kind="ExternalInput").ap()
    f0m = nc.dram_tensor("f0m", [P, FEAT], F16, kind="ExternalInput").ap()
    dwm = nc.dram_tensor("dwm", [P, G], I32, kind="ExternalInput").ap()
    dshm = nc.dram_tensor("dshm", [P, 1], I32, kind="ExternalInput").ap()
    if K > 1:
        frnd = nc.dram_tensor("frnd", [P, (K - 1) * FEAT], F16, kind="ExternalInput").ap()
        wm = nc.dram_tensor("wm", [P, K - 1], mybir.dt.uint8, kind="ExternalInput").ap()
    out = nc.dram_tensor("rowsums", [P, RB * CT], F32, kind="ExternalOutput").ap()

    with tile.TileContext(nc) as tc:
        with (
            tc.tile_pool(name="persist", bufs=1) as pp,
            tc.tile_pool(name="sq", bufs=12) as sqp,
            tc.tile_pool(name="expp", bufs=4) as expp,
            tc.tile_pool(name="psum", bufs=6, space="PSUM") as psp,
        ):
            idx_t = pp.tile([P, G], I32, tag="idx")
            nc.sync.dma_start(out=idx_t[:], in_=idx0)
            dw_t = pp.tile([P, G], I32, tag="dw")
            nc.sync.dma_start(out=dw_t[:], in_=dw)
            dsh_t = pp.tile([P, G], I32, tag="dsh")
            nc.sync.dma_start(out=dsh_t[:], in_=dsh)
            idxm_t = pp.tile([P, 1], I32, tag="idxm")
            nc.sync.dma_start(out=idxm_t[:], in_=idxm)
            dwm_t = pp.tile([P, G], I32, tag="dwm")
            nc.sync.dma_start(out=dwm_t[:], in_=dwm)
            dshm_t = pp.tile([P, 1], I32, tag="dshm")
            nc.sync.dma_start(out=dshm_t[:], in_=dshm)
            f0_t = pp.tile([P, G * FEAT], F16, tag="f0")
            nc.sync.dma_start(out=f0_t[:], in_=f0)
            f0m_t = pp.tile([P, FEAT], F16, tag="f0m")
            nc.sync.dma_start(out=f0m_t[:], in_=f0m)
            if K > 1:
                fr_t = pp.tile([P, (K - 1) * FEAT], F16, tag="fr")
                nc.sync.dma_start(out=fr_t[:], in_=frnd)
                wm_t = pp.tile([P, K - 1], mybir.dt.uint8, tag="wm")
                nc.sync.dma_start(out=wm_t[:], in_=wm)

            prev_out_dma = None
            for _rep in range(repeat):
                # ---- EMA round 0 over the 8 range blocks ------------------------
                g_all = pp.tile([P, G, FEAT], F16, tag="g_all")
                for g in range(G):
                    gather = nc.gpsimd.indirect_dma_start(
                        out=g_all[:, g, :], out_offset=None, in_=p_orig,
                        in_offset=bass.IndirectOffsetOnAxis(ap=idx_t[:, g:g + 1], axis=0))
                    if g == 0 and prev_out_dma is not None:
                        _add_dep_helper(gather.ins, prev_out_dma.ins, sync=True,
                                        reason="serialize-repeat")
                g_flat = g_all[:].rearrange("p a b -> p (a b)")
                for g in range(G):
                    nc.vector.tensor_scalar(out=g_all[:, g, 0:1], in0=g_all[:, g, 0:1],
                                            scalar1=1.0, scalar2=None, op0=OP.mult)
                nc.vector.tensor_scalar(out=f0_t[:, 0:1], in0=f0_t[:, 0:1],
                                        scalar1=1.0, scalar2=None, op0=OP.mult)
                t_all = pp.tile([P, G, FEAT], F16, tag="t_all")
                t_flat = t_all[:].rearrange("p a b -> p (a b)")
                nc.vector.scalar_tensor_tensor(out=t_flat, in0=g_flat, scalar=0.95,
                                               in1=f0_t[:], op0=OP.mult, op1=OP.add)
                ss = pp.tile([P, G], F32, tag="ss")
                for g in range(G):
                    sq = sqp.tile([P, FEAT], F16, name=f"sqb{g}", tag="sqt")
                    nc.scalar.activation(sq[:], t_all[:, g, :], AF.Square,
                                         accum_out=ss[:, g:g + 1])
                nrm = pp.tile([P, G], F32, tag="nrm")
                nc.scalar.activation(nrm[:], ss[:], AF.Sqrt)
                rinv = pp.tile([P, G], F32, tag="rinv")
                nc.vector.reciprocal(rinv[:], nrm[:])
                tmp = pp.tile([P, G], F32, tag="tmp")
                _nr_refine(nc, rinv, tmp, ss)
                v_all = pp.tile([P, G, FEAT], F16, tag="v_all")
                for g in range(G):
                    nc.vector.tensor_scalar(out=v_all[:, g, :], in0=t_all[:, g, :],
                                            scalar1=rinv[:, g:g + 1], scalar2=None,
                                            op0=OP.mult)

                # ---- dedicated multi-occurrence tile: round 0 + rounds ----------
                m_g = pp.tile([P, FEAT], F16, tag="m_g")
                nc.gpsimd.indirect_dma_start(
                    out=m_g[:], out_offset=None, in_=p_orig,
                    in_offset=bass.IndirectOffsetOnAxis(ap=idxm_t[:, 0:1], axis=0))
                nc.vector.tensor_scalar(out=m_g[:, 0:1], in0=m_g[:, 0:1],
                                        scalar1=1.0, scalar2=None, op0=OP.mult)
                nc.vector.tensor_scalar(out=f0m_t[:, 0:1], in0=f0m_t[:, 0:1],
                                        scalar1=1.0, scalar2=None, op0=OP.mult)
                v_m = pp.tile([P, FEAT], F16, tag="v_m")
                t_m = pp.tile([P, FEAT], F16, tag="t_m")
                nc.vector.scalar_tensor_tensor(out=t_m[:], in0=m_g[:], scalar=0.95,
                                               in1=f0m_t[:], op0=OP.mult, op1=OP.add)
                rinv_m = _ema_norm(nc, pp, sqp, t_m[:], "m0")
                nc.vector.tensor_scalar(out=v_m[:], in0=t_m[:],
                                        scalar1=rinv_m[:, 0:1], scalar2=None, op0=OP.mult)
                if K > 1:
                    if _rep == 0:
                        nc.vector.tensor_scalar(out=fr_t[:, 0:1], in0=fr_t[:, 0:1],
                                                scalar1=1.0, scalar2=None, op0=OP.mult)
                        nc.vector.tensor_scalar(out=wm_t[:, 0:1], in0=wm_t[:, 0:1],
                                                scalar1=1.0, scalar2=None, op0=OP.mult)
                    for k in range(1, K):
                        t0 = pp.tile([P, FEAT], F16, tag="t0")
                        nc.vector.scalar_tensor_tensor(
                            out=t0[:], in0=v_m[:], scalar=0.95,
                            in1=fr_t[:, (k - 1) * FEAT:k * FEAT],
                            op0=OP.mult, op1=OP.add)
                        rinv0 = _ema_norm(nc, pp, sqp, t0[:], f"r{k}")
                        nv0 = pp.tile([P, FEAT], F16, tag="nv0")
                        nc.vector.tensor_scalar(out=nv0[:], in0=t0[:],
                                                scalar1=rinv0[:, 0:1], scalar2=None,
                                                op0=OP.mult)
                        nc.vector.copy_predicated(
                            out=v_m[:],
                            mask=wm_t[:, k - 1:k].to_broadcast([P, FEAT]),
                            data=nv0[:])

                # ---- flush: per-range scatters (small APs), multi overwrites ----
                for g in range(G):
                    nc.gpsimd.indirect_dma_start(
                        out=p_w[g],
                        out_offset=bass.IndirectOffsetOnAxis(ap=dw_t[:, g:g + 1], axis=0),
                        in_=v_all[:, g, :], in_offset=None)
                    nc.gpsimd.indirect_dma_start(
                        out=p_shard,
                        out_offset=bass.IndirectOffsetOnAxis(ap=dsh_t[:, g:g + 1], axis=0),
                        in_=v_all[:, g, :], in_offset=None)
                for r in range(G):
                    nc.gpsimd.indirect_dma_start(
                        out=p_w[r],
                        out_offset=bass.IndirectOffsetOnAxis(ap=dwm_t[:, r:r + 1], axis=0),
                        in_=v_m[:], in_offset=None)
                nc.gpsimd.indirect_dma_start(
                    out=p_shard,
                    out_offset=bass.IndirectOffsetOnAxis(ap=dshm_t[:, 0:1], axis=0),
                    in_=v_m[:], in_offset=None)

                # ---- transposes ------------------------------------------------
                lhsT = [pp.tile([P, SHARD], F16, tag=f"lhsT{f}", name=f"lhsT{f}")
                        for f in range(KC)]
                for f in range(KC):
                    nc.sync.dma_start_transpose(
                        lhsT[f][:], p_shard[0:SHARD, P * f:P * (f + 1)])
                PT = [pp.tile([P, NCLS], F16, tag=f"PT{f}", name=f"PT{f}")
                      for f in range(KC)]
                for c in range(CT):
                    r, half = c // 2, c % 2
                    for f in range(KC):
                        nc.sync.dma_start_transpose(
                            PT[f][:, 512 * c:512 * (c + 1)],
                            p_w[r][512 * half:512 * (half + 1), P * f:P * (f + 1)])

                # ---- gram x exp x rowsum ---------------------------------------
                rowsums_t = pp.tile([P, RB * CT], F32, tag="rs")
                for c in range(CT):
                    for rb in range(RB):
                        ps = psp.tile([P, 512], F32, tag="ps")
                        for f in range(KC):
                            nc.tensor.matmul(
                                ps[:], lhsT=lhsT[f][:, P * rb:P * (rb + 1)],
                                rhs=PT[f][:, 512 * c:512 * (c + 1)],
                                start=(f == 0), stop=(f == KC - 1))
                        es = expp.tile([P, 512], F16)
                        slot = rb * CT + c
                        nc.scalar.activation(es[:], ps[:], AF.Exp, scale=1.0 / TEMP,
                                             accum_out=rowsums_t[:, slot:slot + 1])
                prev_out_dma = nc.sync.dma_start(out=out, in_=rowsums_t[:])
    if split_waits:
        _split_multi_waits(nc)
    return nc


def host_prep(features, labels, prototypes):
    feats = np.asarray(features, np.float32)
    labs = np.asarray(labels).reshape(-1).astype(np.int64)
    prot = np.asarray(prototypes, np.float32)

    occ: dict = {}
    for t, l in enumerate(labs.tolist()):
        occ.setdefault(int(l), []).append(t)
    K = max(len(v) for v in occ.values())
    multi = sorted([l for l in occ if len(occ[l]) >= 2], key=lambda l: -len(occ[l]))
    assert len(multi) <= P

    idx0 = np.full((P, G), NCLS, np.int32)
    f0 = np.zeros((P, G, FEAT), np.float32)
    f0[:, :, 0] = 0.05
    dw = np.full((P, G), SHARD, np.int32)
    dsh_all = np.full((NCORES, P, G), SHARD, np.int32)
    for r in range(G):
        Lr = sorted(l for l in occ if l // SHARD == r)
        assert len(Lr) <= P, f"range {r} has {len(Lr)} labels"
        for p, l in enumerate(Lr):
            idx0[p, r] = l
            f0[p, r, :] = 0.05 * feats[occ[l][0]]
            dw[p, r] = l - r * SHARD
            dsh_all[r, p, r] = l - r * SHARD   # block r hits shard r only
    idxm = np.full((P, 1), NCLS, np.int32)
    f0m = np.zeros((P, FEAT), np.float32)
    f0m[:, 0] = 0.05
    dwm = np.full((P, G), SHARD, np.int32)
    dshm_all = np.full((NCORES, P, 1), SHARD, np.int32)
    nr = max(K - 1, 1)
    frnd = np.zeros((P, nr, FEAT), np.float32)
    wmv = np.zeros((P, nr), np.uint8)
    for p, l in enumerate(multi):
        idxm[p, 0] = l
        f0m[p, :] = 0.05 * feats[occ[l][0]]
        r = l // SHARD
        dwm[p, r] = l - r * SHARD
        dshm_all[r, p, 0] = l - r * SHARD
        for k in range(1, len(occ[l])):
            frnd[p, k - 1, :] = 0.05 * feats[occ[l][k]]
            wmv[p, k - 1] = 1

    p16 = prot.astype(np.float16)
    zrow = np.zeros((1, FEAT), np.float16)
    in_maps = []
    for c in range(NCORES):
        m = {
            "p_orig": np.vstack([p16, zrow]),
            "p_shard": np.vstack([p16[c * SHARD:(c + 1) * SHARD], zrow]),
            "f0": f0.reshape(P, G * FEAT).astype(np.float16),
            "idx0": idx0, "dw": dw, "dsh": dsh_all[c],
            "idxm": idxm, "f0m": f0m.astype(np.float16),
            "dwm": dwm, "dshm": dshm_all[c],
        }
        for r in range(G):
            m[f"p_w{r}"] = np.vstack([p16[r * SHARD:(r + 1) * SHARD], zrow])
        if K > 1:
            m["frnd"] = frnd.reshape(P, nr * FEAT).astype(np.float16)
            m["wm"] = wmv
        in_maps.append(m)
    return in_maps, K


def reduce_output(results) -> np.ndarray:
    S = np.zeros(NCLS, np.float64)
    for c in range(NCORES):
        rs = np.asarray(results[c]["rowsums"], np.float64)
        per_row = rs.reshape(P, RB, CT).sum(-1)
        S[c * SHARD:(c + 1) * SHARD] = per_row.T.reshape(-1)
    mean_prob_neg = np.log((S - math.exp(1.0 / TEMP)) / (NCLS - 1))
    return np.array((TEMP / BASE_TEMP) * mean_prob_neg.mean(), dtype=np.float32)


def kernel(features, labels, prototypes):
    in_maps, K = host_prep(features, labels, prototypes)
    key = (K, 1)
    if key not in _PROGRAM_CACHE:
        _PROGRAM_CACHE[key] = build_program(K, repeat=1)
    nc = _PROGRAM_CACHE[key]
    r = run_bass_kernel_spmd(nc, in_maps, list(range(NCORES)), trace=False)
    return reduce_output(r.results)
